# revision 2
# baseline (speedup 1.0000x reference)
"""BottleneckAttn Trainium2 kernel, v2.

Full inputs -> full output. 8-way head-parallel (one (batch, head) pair per
NeuronCore). Per core:

Everything runs in w-major spatial order: n' = w*64 + g (w = image col,
g = image row), applied to queries AND keys via a host-side transpose of x.

  l1^T[m, n] = sum_d k[d,m] q'[d,n] + XW^T[w'(m), n]      (PE, fp16 in)
  pt = exp(l1^T - 4)                                       (ACT, fp32r out)
  ptw = pt * EH2[m%64-dup, n],  EH2 = exp(XH^T)            (DVE/Pool, fp16 out)
  av[n, dv|S] += ptw[:, nchunk]^T @ [v | 1]                (PE, 65-col fp16)
  out[n, dv] = av[n, dv] / av[n, 64]                       (DVE recip + mul)

The W-bias rides in the mm1 contraction via a 0/1 block selector E_w stacked
under k (contraction 128 = 64 k-dims + 64 E_w rows, E_w[j, m]=[m//64==j]);
the H-bias is folded multiplicatively into P after the exp (the exp(-4)
rescale cancels in the normalize). w-major makes EH2 contiguous for the mult
(h'(m) = m%64 is partition-periodic) -- the one strided pass (XH^T psum ->
n'-ordered staging) happens once in setup, not per logit tile. The AV matmul
runs transposed (n on partitions) so the softmax denominator lands as a
per-partition scalar.
"""

import sys

if "/opt/trn_rl_repo" not in sys.path:
    sys.path.insert(0, "/opt/trn_rl_repo")

import numpy as np

import concourse.bass as bass
import concourse.tile as tile
from concourse import bacc, mybir
from concourse.bass_utils import run_bass_kernel_spmd

B, C, H, W = 2, 256, 64, 64
NH, D = 4, 64
HW = H * W   # 4096
NP = 4       # n pairs (1024 n each)
NMC = 32     # m chunks of 128
FP32 = mybir.dt.float32
FP32R = mybir.dt.float32r
FP16 = mybir.dt.float16
AF = mybir.ActivationFunctionType

_prog = None


def _build():
    nc = bacc.Bacc("TRN2", target_bir_lowering=False, debug=False)

    x_d = nc.dram_tensor("x", [2, 128, HW], FP16, kind="ExternalInput").ap()
    wqk_d = nc.dram_tensor("wqk", [2, 128, 128], FP16, kind="ExternalInput").ap()
    wv_d = nc.dram_tensor("wv", [2, 128, D], FP16, kind="ExternalInput").ap()
    hrel_d = nc.dram_tensor("hrel", [64, 127], FP16, kind="ExternalInput").ap()
    wrel_d = nc.dram_tensor("wrel", [64, 127], FP16, kind="ExternalInput").ap()
    eh_d = nc.dram_tensor("eh", [64, HW], FP16, kind="ExternalInput").ap()
    onesv_d = nc.dram_tensor("onesv", [128, NMC], FP16, kind="ExternalInput").ap()
    out_d = nc.dram_tensor("out", [HW, D], FP32, kind="ExternalOutput").ap()

    with tile.TileContext(nc) as tc:
        with (
            tc.tile_pool(name="const", bufs=1) as constp,
            tc.tile_pool(name="big", bufs=1) as bigp,
            tc.tile_pool(name="ptp", bufs=4) as ptp,
            tc.tile_pool(name="ptwp", bufs=6) as ptwp,
            tc.tile_pool(name="outp", bufs=4) as outp,
            tc.tile_pool(name="pp_psum", bufs=2, space="PSUM") as pp_psum,
            tc.tile_pool(name="aux_psum", bufs=2, space="PSUM") as aux_psum,
            tc.tile_pool(name="av_psum", bufs=2, space="PSUM") as av_psum,
        ):
            # ---------------- sbuf tiles ----------------
            x_sb = bigp.tile([128, 2, 64, 64], FP16)  # [c, t, w, g]
            wqk_sb = constp.tile([128, 2, 128], FP16)
            wv_sb = constp.tile([128, 2, D], FP16)
            hrel_sb = constp.tile([64, 127], FP16)
            wrel_sb = constp.tile([64, 127], FP16)
            keh = bigp.tile([128, HW], FP16)        # rows 0:64 k, 64:128 E_w
            rhs1 = bigp.tile([128, 64, 64], FP16)   # [p, w, g]: q' | XW^T
            eh2 = bigp.tile([128, HW], FP16)        # exp(XH^T) dup'd, n'-order
            xh_st = bigp.tile([64, 64, 64], FP32)   # XH^T staging [j, w, g]
            v_t = bigp.tile([128, NMC, D + 1], FP16)
            recips = bigp.tile([128, NP, 8], FP32)
            bias4 = constp.tile([128, 1], FP32)
            nc.gpsimd.memset(bias4[:, :], -4.0)

            for t in range(2):
                nc.sync.dma_start(out=wqk_sb[:, t, :], in_=wqk_d[t])
                nc.sync.dma_start(out=wv_sb[:, t, :], in_=wv_d[t])
                pass
            qs = [nc.sync, nc.scalar, nc.gpsimd]
            for cb in range(8):
                xsl = slice(cb * 512, (cb + 1) * 512)
                for t in range(2):
                    qs[(2 * cb + t) % 3].dma_start(
                        out=x_sb[:, t, 8 * cb:8 * (cb + 1), :],
                        in_=x_d[t, :, xsl])
            nc.scalar.dma_start(out=hrel_sb[:, :], in_=hrel_d[:, :])
            nc.gpsimd.dma_start(out=wrel_sb[:, :], in_=wrel_d[:, :])
            nc.sync.dma_start(out=keh[64:128, :], in_=eh_d[:, :])
            nc.sync.dma_start(out=v_t[:, :, D], in_=onesv_d[:, :])

            def lead_ps(idx, name):
                # alternate aux/pp pools so 4 lead batches are in flight
                if idx % 2 == 0:
                    return aux_psum.tile([128, 512], FP32, name=name,
                                         tag="aux")
                return pp_psum.tile([128, 1024], FP32, name=name,
                                    tag="pp")[:, 0:512]

            # ---- phase 1: q'|k projections ----
            for nb in range(8):
                ps = lead_ps(nb, "psqk")
                for t in range(2):
                    nc.tensor.matmul(
                        ps[:, :], wqk_sb[:, t, :],
                        x_sb[:, t, 8 * nb:8 * (nb + 1), :],
                        start=(t == 0), stop=(t == 1),
                    )
                nc.vector.tensor_copy(
                    rhs1[0:64, 8 * nb:8 * (nb + 1), :], ps[0:64, :]
                )
                nc.scalar.copy(keh[0:64, nb * 512:(nb + 1) * 512],
                               ps[64:128, :])

            # ---- phase 4: XH^T -> staging (strided) ----
            for gb in range(8):
                ps = lead_ps(gb, "psh")
                for i in range(8):
                    g = 8 * gb + i
                    nc.tensor.matmul(
                        ps[0:64, 64 * i:64 * (i + 1)],
                        hrel_sb[:, 63 - g:127 - g],
                        rhs1[0:64, :, g],
                        start=True, stop=True,
                    )
                xdst = xh_st[:, :, 8 * gb:8 * (gb + 1)].transpose([0, 2, 1])
                if gb % 2 == 0:
                    nc.vector.tensor_copy(xdst, ps[0:64, :])
                else:
                    nc.scalar.copy(xdst, ps[0:64, :])

            # ---- phase 3: XW^T -> rhs1[64:128] ----
            def ph3_batch(wbt, dve):
                psw = lead_ps(wbt, "psw") if not dve else aux_psum.tile(
                    [128, 512], FP32, name="psw", tag="aux")
                for i in range(8):
                    ww = 8 * wbt + i
                    nc.tensor.matmul(
                        psw[0:64, 64 * i:64 * (i + 1)],
                        wrel_sb[:, 63 - ww:127 - ww],
                        rhs1[0:64, ww, :],
                        start=True, stop=True,
                    )
                dst = rhs1[64:128, 8 * wbt:8 * (wbt + 1), :]
                if dve:
                    nc.vector.tensor_copy(dst, psw[0:64, :])
                else:
                    nc.scalar.copy(dst, psw[0:64, :])

            for wbt in range(2):
                ph3_batch(wbt, False)

            # eh2 = exp(XH^T - 4) (the -4 cancels in the normalize; keeping
            # it here instead of in the 128 hot exps leaves those bias-free)
            for c in range(4):
                nc.scalar.activation(
                    eh2[0:64, 1024 * c:1024 * (c + 1)],
                    xh_st[:, 16 * c:16 * (c + 1), :], AF.Exp,
                    bias=bias4[0:64, 0:1],
                )
                nc.vector.tensor_copy(
                    eh2[64:128, 1024 * c:1024 * (c + 1)],
                    eh2[0:64, 1024 * c:1024 * (c + 1)],
                )


            # ---- phase 2: v (4 m-chunks per psum tile, batched cast) ----
            def ph2_batch(vb):
                ps = aux_psum.tile([128, 512], FP32, name="psv", tag="aux")
                for j in range(4):
                    vmc = 4 * vb + j
                    for t in range(2):
                        nc.tensor.matmul(
                            ps[:, 64 * j:64 * (j + 1)],
                            x_sb[:, t, 2 * vmc:2 * vmc + 2, :],
                            wv_sb[:, t, :], start=(t == 0), stop=(t == 1),
                        )
                nc.vector.tensor_copy(
                    v_t[:, 4 * vb:4 * (vb + 1), 0:D], ps[:, 0:256]
                )

            for vb in range(2):
                ph2_batch(vb)

            # ---------------- phase 5: main loop (flat, no p barrier) ----
            SKEW = 2
            TOT = NP * NMC
            ptws = {}
            avs_by_p = {}
            pending_norm = []

            def emit_norm():
                pj, i, avs = pending_norm.pop(0)
                av = avs[i // 4]
                c0 = 65 * (i % 4)
                nc.vector.reciprocal(
                    recips[:, pj, i:i + 1], av[:, c0 + 64:c0 + 65]
                )
                ot = outp.tile([128, 64], FP32, name="ot")
                nc.vector.tensor_scalar_mul(
                    ot[:, :], av[:, c0:c0 + 64], recips[:, pj, i:i + 1]
                )
                nsl = slice(pj * 1024 + 128 * i, pj * 1024 + 128 * (i + 1))
                oq = [nc.sync, nc.scalar, nc.gpsimd][i % 3]
                oq.dma_start(out=out_d[nsl, :], in_=ot[:, :])

            for s in range(TOT + SKEW):
                if pending_norm:
                    emit_norm()
                if s < TOT:
                    p, mc = divmod(s, NMC)
                    n0 = p * 1024
                    if p == 0 and mc % 2 == 1 and 7 <= mc <= 17:
                        ph2_batch(2 + (mc - 7) // 2)
                    if p == 0 and mc % 2 == 1 and 19 <= mc <= 29:
                        ph3_batch(2 + (mc - 19) // 2, True)
                    pp = pp_psum.tile([128, 1024], FP32, name="pp", tag="pp")
                    for half in range(2):
                        nc.tensor.matmul(
                            pp[:, 512 * half:512 * (half + 1)],
                            keh[:, mc * 128:(mc + 1) * 128],
                            rhs1[:, 8 * (2 * p + half):
                                 8 * (2 * p + half + 1), :],
                            start=True, stop=True,
                        )
                    pt = ptp.tile([128, 1024], FP16, name="pt")
                    nc.scalar.activation(pt[:, :], pp[:, :], AF.Exp)
                    ptw = ptwp.tile([128, 1024], FP16, name="ptw")
                    for half in range(2):
                        hsl = slice(512 * half, 512 * (half + 1))
                        nsl = slice(n0 + 512 * half, n0 + 512 * (half + 1))
                        # 6:2 DVE:Pool split (Pool TT is ~2.3x slower),
                        # spread so Pool never takes 2 halves in a row
                        eng = (nc.gpsimd if (2 * mc + half) % 8 in (3, 6)
                               else nc.vector)
                        eng.tensor_mul(ptw[:, hsl], pt[:, hsl], eh2[:, nsl])
                    ptws[s] = ptw
                if s >= SKEW:
                    sj = s - SKEW
                    pj, j = divmod(sj, NMC)
                    if j == 0:
                        avp = av_psum if pj % 2 == 0 else aux_psum
                        avtag = "av" if pj % 2 == 0 else "aux"
                        avs_by_p[pj] = [
                            avp.tile([128, 512], FP32, name=f"av{i}",
                                     tag=avtag)
                            for i in range(2)
                        ]
                    avs = avs_by_p[pj]
                    ptwj = ptws.pop(sj)
                    for i in range(8):
                        # start=True clears has_written bits for the WHOLE
                        # bank: only the first series per bank may use it.
                        # The others overwrite at j==0 via the cleared bits.
                        nc.tensor.matmul(
                            avs[i // 4][:, 65 * (i % 4):65 * (i % 4) + 65],
                            ptwj[:, 128 * i:128 * (i + 1)],
                            v_t[:, j, :],
                            start=(j == 0 and i % 4 == 0),
                            stop=(j == NMC - 1),
                            skip_group_check=True,
                        )
                    if j == NMC - 1:
                        for i in range(8):
                            pending_norm.append((pj, i, avs))
            while pending_norm:
                emit_norm()

    nc.finalize()
    return nc


def _get_program():
    global _prog
    if _prog is None:
        _prog = _build()
    return _prog


def _make_in_maps(x, qkv_w, height_rel, width_rel):
    x = np.asarray(x, dtype=np.float32)
    qkv_w = np.asarray(qkv_w, dtype=np.float32)
    height_rel = np.asarray(height_rel, dtype=np.float32)
    width_rel = np.asarray(width_rel, dtype=np.float32)

    hrel_t = (height_rel * np.float32(8.0)).T.astype(np.float16)  # (64, 127)
    wrel_t = (width_rel * np.float32(8.0)).T.astype(np.float16)

    eh = np.zeros((64, HW), dtype=np.float16)
    for j in range(64):
        eh[j, j * 64:(j + 1) * 64] = 1.0

    # w-major spatial order: n' = w*64 + g
    xT = np.ascontiguousarray(
        x.reshape(B, C, H, W).transpose(0, 1, 3, 2).reshape(B, C, HW))
    x16 = xT.astype(np.float16)
    in_maps = []
    for core in range(8):
        b, h = divmod(core, 4)
        wq = qkv_w[D * h:D * (h + 1)] * np.float32(0.125)
        wk = qkv_w[C + D * h:C + D * (h + 1)]
        wv = qkv_w[2 * C + D * h:2 * C + D * (h + 1)]
        wqk = np.concatenate([wq, wk], axis=0)              # (128, 256)
        in_maps.append({
            "x": np.ascontiguousarray(x16[b].reshape(2, 128, HW)),
            "wqk": np.ascontiguousarray(
                wqk.T.reshape(2, 128, 128).astype(np.float16)),
            "wv": np.ascontiguousarray(
                wv.T.reshape(2, 128, D).astype(np.float16)),
            "hrel": np.ascontiguousarray(hrel_t),
            "wrel": np.ascontiguousarray(wrel_t),
            "eh": eh,
            "onesv": np.ones((128, NMC), dtype=np.float16),
        })
    return in_maps


def _assemble(results):
    out = np.empty((B, C, H, W), dtype=np.float32)
    for core in range(8):
        b, h = divmod(core, 4)
        o = np.asarray(results[core]["out"], dtype=np.float32)  # (n'=w*64+g, dv)
        out[b, D * h:D * (h + 1)] = o.reshape(W, H, D).transpose(2, 1, 0)
    return out


def kernel(x, qkv_w, height_rel, width_rel):
    nc = _get_program()
    in_maps = _make_in_maps(x, qkv_w, height_rel, width_rel)
    res = run_bass_kernel_spmd(nc, in_maps, list(range(8)))
    return _assemble(res.results)
